# revision 26
# baseline (speedup 1.0000x reference)
"""Trainium2 Bass kernel for nn_DownSample (FPS + gathers + strided conv + BN + GELU).

Data-parallel over batch: 64 global batches -> 8 cores x 8 batches.
Self-contained: hardcodes all shapes; builds + compiles + runs via
run_bass_kernel_spmd on cores 0-7; returns full (unsharded) outputs.

Per-core plan:
  - FPS over stroke coords via a precomputed pairwise half-distance matrix
    T[n,n'] = 0.5|x_n|^2 + 0.5|x_n'|^2 - <x_n,x_n'> (d = 2*T), replicated
    across 16-partition groups (batch g on partitions 16g..16g+15) so the
    per-iteration distance-column gather is one gpsimd ap_gather (rows of a
    [128 x 32] half-row view; ids 2j+parity).
  - argmax per batch via DVE max/max_index (exact first-occurrence).
  - After every second FPS iteration, the two new strokes' dense data is
    gathered for all batches in one cross-batch ap_gather, rounded to
    float32r into zero-padded 66-wide blocks, and the strided conv for
    those two output strokes runs on the PE at 1 cycle/row, pipelined
    behind the FPS chain. Index vectors are built with tiny bf16 matmuls.
  - Conv outputs (pre-BN, +bias) stream to an HBM scratch; per-channel
    sum/sumsq accumulate on the fly; a 2KB AllReduce produces the global
    batch stats; y is read back and BN+GELU is applied in one ACT pass
    (scale/bias fused into Gelu) while un-permuting, then written out.
"""
import sys

sys.path.insert(0, "/opt/trn_rl_repo")

import numpy as np

import concourse.bacc as bacc
import concourse.mybir as mybir
import concourse.tile as tile
from concourse import tile_rust
from concourse import bass
from concourse.bass_utils import run_bass_kernel_spmd

F32 = mybir.dt.float32
F32R = mybir.dt.float32r
BF16 = mybir.dt.bfloat16
I16 = mybir.dt.int16
U16 = mybir.dt.uint16
ALU = mybir.AluOpType
ACTF = mybir.ActivationFunctionType
AXL = mybir.AxisListType

B = 8          # batches per core
NSTK = 64      # strokes
NPNT = 64      # points per stroke
CO = 32        # coordinate embedding
NH = 32        # sampled strokes
CIN = 128
COUT = 256
SP = 256
T = 32         # conv output positions per stroke
PB = NPNT + 2  # padded conv input block (zero, x0..x63, zero)
BN_EPS = 1e-5
M_GLOBAL = 64 * NH * T  # 65536

_CACHE = {}


def _ap(x, off_delta, dims):
    return bass.AP(x.tensor, x.offset + off_delta, dims)


def _build():
    nc = bacc.Bacc(None, target_bir_lowering=False)

    sparse_d = nc.dram_tensor("sparse_fea", [B, SP, NSTK], F32, kind="ExternalInput")
    dense_d = nc.dram_tensor("dense_fea", [B, CIN, NSTK * NPNT], F32, kind="ExternalInput")
    stk_d = nc.dram_tensor("stk_coor", [B, NSTK, CO], F32, kind="ExternalInput")
    convw_d = nc.dram_tensor("conv_w", [COUT, CIN, 1, 3], F32, kind="ExternalInput")
    convb_d = nc.dram_tensor("conv_b", [COUT], F32, kind="ExternalInput")
    gam_d = nc.dram_tensor("bn_gamma", [COUT], F32, kind="ExternalInput")
    bet_d = nc.dram_tensor("bn_beta", [COUT], F32, kind="ExternalInput")

    sparse_o = nc.dram_tensor("sparse_out", [B, SP, NH], F32, kind="ExternalOutput")
    dense_o = nc.dram_tensor("dense_out", [B, COUT, NH * T], F32, kind="ExternalOutput")
    stk_o = nc.dram_tensor("stk_out", [B, NH, CO], F32, kind="ExternalOutput")

    ident_c = nc.inline_tensor(np.eye(128, dtype=np.float32), "ident128")
    permrow_c = nc.inline_tensor(
        (64.0 * (np.arange(128) % 8)).astype(np.float32).reshape(1, 128), "permrow")
    ones_c = nc.inline_tensor(np.ones((1, 1), np.float32), "ones11")
    boffBf_c = nc.inline_tensor(
        (64.0 * (np.arange(128) // 16)).astype(np.float32).reshape(128, 1), "boffBf")
    parity_c = nc.inline_tensor(
        (np.arange(128) % 2).astype(np.float32).reshape(128, 1), "parity")
    # rank-pair selectors: out[0,n] += in[16*(n%8), r] for rows with (n%16)//8==r
    _n = np.arange(128)
    _P0 = np.zeros((128, 128), np.float32)
    _P1 = np.zeros((128, 128), np.float32)
    _m0 = (_n % 16) // 8 == 0
    _P0[16 * (_n[_m0] % 8), _n[_m0]] = 1.0
    _P1[16 * (_n[~_m0] % 8), _n[~_m0]] = 1.0
    psel0_c = nc.inline_tensor(_P0, "psel0")
    psel1_c = nc.inline_tensor(_P1, "psel1")

    GXW = NSTK * NSTK + NSTK

    with tile.TileContext(nc) as tc:
        with tc.tile_pool(name="dram", bufs=1, space="DRAM") as dpool, \
             tc.tile_pool(name="persist", bufs=1) as pp, \
             tc.tile_pool(name="work", bufs=2) as wp:

            g_hbm = dpool.tile([B, GXW], F32)
            fps_dram = dpool.tile([B, NH], I16)
            fps2_dram = dpool.tile([B, NH], I16)
            y_hbm = dpool.tile([256, 16 * 512], F32)
            cc_in = dpool.tile([128, 4], F32)
            cc_out = dpool.tile([128, 4], F32)

            ident_sb = pp.tile([128, 128], F32)
            nc.sync.dma_start(ident_sb[:], ident_c[:, :])
            ones_sb = pp.tile([1, 1], F32)
            nc.sync.dma_start(ones_sb[:], ones_c[:, :])
            boffBf_sb = pp.tile([128, 1], F32)
            nc.sync.dma_start(boffBf_sb[:], boffBf_c[:, :])
            parity_sb = pp.tile([128, 1], F32)
            nc.sync.dma_start(parity_sb[:], parity_c[:, :])

            # bf16 index-build constants (values exact in bf16)
            psel0_b = pp.tile([128, 128], BF16)
            psel1_b = pp.tile([128, 128], BF16)
            permrow_b = pp.tile([1, 128], BF16)
            ones_b = pp.tile([1, 1], BF16)
            with tc.tile_pool(name="cload", bufs=2) as clp:
                c1 = clp.tile([128, 128], F32, tag="c1")
                nc.sync.dma_start(c1[:], psel0_c[:, :])
                nc.scalar.copy(psel0_b[:], c1[:])
                c2 = clp.tile([128, 128], F32, tag="c1")
                nc.sync.dma_start(c2[:], psel1_c[:, :])
                nc.scalar.copy(psel1_b[:], c2[:])
                c3 = clp.tile([1, 128], F32, tag="c3")
                nc.sync.dma_start(c3[:], permrow_c[:, :])
                nc.scalar.copy(permrow_b[:], c3[:])
                nc.scalar.copy(ones_b[:], ones_sb[:])

            convb_sb = pp.tile([128, 2], F32)
            gam_sb = pp.tile([128, 2], F32)
            bet_sb = pp.tile([128, 2], F32)
            for h in range(2):
                nc.sync.dma_start(convb_sb[:, h:h + 1], convb_d[h * 128:(h + 1) * 128])
                nc.sync.dma_start(gam_sb[:, h:h + 1], gam_d[h * 128:(h + 1) * 128])
                nc.sync.dma_start(bet_sb[:, h:h + 1], bet_d[h * 128:(h + 1) * 128])

            # conv weights -> 6 stationary f32r tiles [i=128, o=128]
            wt_sb = pp.tile([128, 6, 128], F32R)
            with tc.tile_pool(name="psW", bufs=2, space="PSUM") as psW, \
                 tc.tile_pool(name="wload", bufs=1) as wlp:
                wsb = wlp.tile([128, 2, 3 * CIN], F32)
                for h in range(2):
                    nc.sync.dma_start(
                        wsb[:, h, :],
                        convw_d[h * 128:(h + 1) * 128].rearrange("o i u k -> o (i u k)"))
                for h in range(2):
                    for k in range(3):
                        wps = psW.tile([128, 128], F32, tag="wps")
                        w_h = wsb[:, h, :]
                        win = _ap(w_h, k, [list(w_h.ap[0]), [3, CIN]])
                        nc.tensor.transpose(wps[:], win, ident_sb[:])
                        nc.scalar.copy(wt_sb[:, h * 3 + k, :], wps[:])

            # stroke coords: Gram + 0.5*xsq -> g_hbm
            with tc.tile_pool(name="psG", bufs=2, space="PSUM") as psG, \
                 tc.tile_pool(name="gwork", bufs=2) as gw:
                stknat = gw.tile([64, B, CO], F32, tag="stknat")
                for b in range(B):
                    nc.sync.dma_start(stknat[:, b, :], stk_d[b])
                stkT = gw.tile([CO, B * NSTK], F32, tag="stkT")
                for b in range(B):
                    tps = psG.tile([CO, NSTK], F32, tag="tps")
                    nc.tensor.transpose(tps[:], stknat[:, b, :], ident_sb[:64, :64])
                    nc.scalar.copy(stkT[:, b * NSTK:(b + 1) * NSTK], tps[:])
                for b in range(B):
                    gps = psG.tile([NSTK, NSTK], F32, tag="gps")
                    sl = stkT[:, b * NSTK:(b + 1) * NSTK]
                    nc.tensor.matmul(gps[:], sl, sl, start=True, stop=True)
                    gsb = gw.tile([NSTK, NSTK], F32, tag="gsb")
                    nc.scalar.copy(gsb[:], gps[:])
                    nc.sync.dma_start(g_hbm[b:b + 1, :NSTK * NSTK], gsb[:])
                    sq = gw.tile([64, CO], F32, tag="sq")
                    nc.vector.tensor_tensor(sq[:], stknat[:, b, :], stknat[:, b, :], ALU.mult)
                    xs = gw.tile([64, 1], F32, tag="xs")
                    nc.vector.tensor_reduce(xs[:], sq[:], AXL.X, ALU.add)
                    nc.vector.tensor_scalar_mul(xs[:], xs[:], 0.5)
                    nc.sync.dma_start(g_hbm[b:b + 1, NSTK * NSTK:], xs[:])

            # ---------------- big scope: FPS + gathers + conv ----------------
            with tc.tile_pool(name="big", bufs=1) as bigp:
                # replicate G and xsq/2 across 16-partition groups; T in place
                txs = bigp.tile([128, GXW], F32)
                tfull = txs[:, :]
                ta = _ap(tfull, 0, [list(tfull.ap[0]), [NSTK, NSTK], [1, NSTK]])
                xa = _ap(tfull, NSTK * NSTK, [list(tfull.ap[0]), [1, NSTK]])
                last_trep = None
                for r in range(16):
                    last_trep = nc.sync.dma_start(
                        _ap(tfull, r * tfull.ap[0][0],
                            [[16 * tfull.ap[0][0], B], [1, GXW]]),
                        g_hbm[:, :])
                xsq_n = _ap(xa, 0, [list(xa.ap[0]), [1, NSTK], [0, NSTK]])
                xsq_np = _ap(xa, 0, [list(xa.ap[0]), [0, NSTK], [1, NSTK]])
                nc.vector.scalar_tensor_tensor(
                    ta, ta, -1.0, xsq_n, ALU.mult, ALU.add)
                nc.vector.tensor_tensor(ta, ta, xsq_np, ALU.add)
                # half-row view for the FPS column gather
                t_half = _ap(ta, 0, [list(ta.ap[0]), [NSTK // 2, 2 * NSTK],
                                     [1, NSTK // 2]])

                # dense features [i, (b,s), p]; issue after the T-path DMAs
                dall = bigp.tile([128, B * NSTK, NPNT], F32)
                for b in range(B):
                    dv = dense_d[b].rearrange("i (s p) -> i s p", p=NPNT)
                    half = NSTK // 2
                    d1 = nc.sync.dma_start(
                        dall[:, b * NSTK:b * NSTK + half, :], dv[:, :half, :])
                    if b == 0:
                        tile_rust.add_dep_helper(
                            d1.ins, last_trep.ins, sync=False,
                            reason="T replication drains the queue before bulk dense")
                    nc.scalar.dma_start(
                        dall[:, b * NSTK + half:(b + 1) * NSTK, :], dv[:, half:, :])

                # FPS state
                dists = pp.tile([128, NSTK], F32)
                nc.vector.tensor_scalar_mul(
                    dists[:], _ap(ta, 0, [list(ta.ap[0]), [NSTK, NSTK]]), 2.0)
                ix8buf = pp.tile([128, 8 * NH], U16)
                nc.vector.memset(ix8buf[:], 0)
                mx8 = pp.tile([128, 8], F32)
                tcol = pp.tile([128, 16, NSTK // 2], F32)
                sums = pp.tile([128, 2, 16], F32)
                sqs = pp.tile([128, 2, 16], F32)
                sqscr = pp.tile([128, 512], F32)

                with tc.tile_pool(name="temp", bufs=2) as tpool, \
                     tc.tile_pool(name="psC", bufs=3, space="PSUM") as psC, \
                     tc.tile_pool(name="psJ", bufs=2, space="PSUM") as psJ, \
                     tc.tile_pool(name="psI", bufs=2, space="PSUM") as psI, \
                     tc.tile_pool(name="ystage", bufs=2) as ysp:
                    for i in range(NH):
                        if i >= 1:
                            nc.vector.max(mx8[:], dists[:])
                            nc.vector.max_index(
                                ix8buf[:, 8 * i:8 * i + 8], mx8[:], dists[:])
                            # half-row ids {2j, 2j+1} by partition parity
                            tji = wp.tile([128, 1], I16, tag="tji")
                            nc.vector.scalar_tensor_tensor(
                                tji[:], ix8buf[:, 8 * i:8 * i + 1], 2.0,
                                parity_sb[:], ALU.mult, ALU.add)
                            nc.gpsimd.ap_gather(
                                tcol[:], t_half, tji[:], channels=128,
                                num_elems=2 * NSTK, d=NSTK // 2, num_idxs=16)
                            nc.vector.scalar_tensor_tensor(
                                dists[:],
                                _ap(tcol[:, :, :], 0, [list(tcol[:, :, :].ap[0]),
                                                       [1, NSTK]]),
                                2.0, dists[:], ALU.mult, ALU.min)

                        if i % 2 == 1:
                            g = i // 2
                            # pair ids: idx[p] = 64*(p%8) + j^{(p%16)//8}_{p%8}
                            jf2 = wp.tile([128, 2], BF16, tag="jf2")
                            ib = ix8buf[:, :]
                            nc.vector.tensor_copy(
                                jf2[:], _ap(ib, 8 * (i - 1), [list(ib.ap[0]), [8, 2]]))
                            jrow_ps = psJ.tile([1, 128], F32, tag="jrow")
                            nc.tensor.matmul(jrow_ps[:], jf2[:, 0:1], psel0_b[:],
                                             start=True, stop=False)
                            nc.tensor.matmul(jrow_ps[:], jf2[:, 1:2], psel1_b[:],
                                             start=False, stop=True)
                            jrow_sb = wp.tile([1, 128], BF16, tag="jrowsb")
                            nc.scalar.copy(jrow_sb[:], jrow_ps[:])
                            idx_ps = psI.tile([128, 1], F32, tag="idxps")
                            nc.tensor.matmul(
                                idx_ps[:], jrow_sb[:], ones_b[:], start=True, stop=False)
                            nc.tensor.matmul(
                                idx_ps[:], permrow_b[:], ones_b[:], start=False, stop=True)
                            idxd = wp.tile([128, 1], I16, tag="idxd")
                            nc.vector.tensor_copy(idxd[:], idx_ps[:])

                            temp = tpool.tile([128, 16, NPNT], F32, tag="temp")
                            nc.gpsimd.ap_gather(
                                temp[:], dall[:], idxd[:],
                                channels=128, num_elems=B * NSTK, d=NPNT, num_idxs=16)
                            # round into zero-padded f32r blocks [z, x0..x63, z]
                            tempr = tpool.tile([128, 16, PB], F32R, tag="tempr")
                            tr = tempr[:, :, :]
                            nc.vector.memset(
                                _ap(tr, 0, [list(tr.ap[0]), [PB, 16],
                                            [PB - 1, 2]]).bitcast(F32), 0)
                            nc.scalar.copy(
                                _ap(tr, 1, [list(tr.ap[0]), [PB, 16], [1, NPNT]]),
                                temp[:])
                            for h in range(2):
                                cps = psC.tile([128, 512], F32, tag="cps")
                                ca = cps[:, :]
                                for k in range(3):
                                    nc.tensor.matmul(
                                        _ap(ca, 0, [list(ca.ap[0]), [2 * T, B],
                                                    [T, 2], [1, T]]),
                                        wt_sb[:, h * 3 + k, :],
                                        _ap(tr, k, [list(tr.ap[0]), [PB, B],
                                                    [8 * PB, 2], [2, T]]),
                                        start=(k == 0), stop=(k == 2),
                                        skip_group_check=(k > 0))
                                yst = ysp.tile([128, 512], F32, tag="yst")
                                nc.scalar.activation(
                                    yst[:], cps[:], ACTF.Identity,
                                    bias=convb_sb[:, h:h + 1], scale=1.0,
                                    accum_out=sums[:, h, g:g + 1])
                                nc.vector.scalar_tensor_tensor(
                                    sqscr[:], yst[:], 1.0, yst[:], ALU.mult, ALU.mult,
                                    accum_out=sqs[:, h, g:g + 1])
                                nc.sync.dma_start(
                                    y_hbm[h * 128:(h + 1) * 128, g * 512:(g + 1) * 512],
                                    yst[:])

                # ---- BN stats + allreduce ----
                stats = pp.tile([128, 4], F32)
                for h in range(2):
                    nc.vector.tensor_reduce(stats[:, h:h + 1], sums[:, h, :], AXL.X, ALU.add)
                    nc.vector.tensor_reduce(stats[:, 2 + h:3 + h], sqs[:, h, :], AXL.X, ALU.add)
                nc.gpsimd.dma_start(cc_in[:, :], stats[:])
                nc.gpsimd.collective_compute(
                    "AllReduce", ALU.add, replica_groups=[list(range(8))],
                    ins=[cc_in[:, :]], outs=[cc_out[:, :]])
                gstats = pp.tile([128, 4], F32)
                nc.gpsimd.dma_start(gstats[:], cc_out[:, :])

                # ---- post-loop index tables ----
                fps_rep = pp.tile([128, NH], I16)
                ia = ix8buf[:, :]
                nc.vector.tensor_copy(fps_rep[:], _ap(ia, 0, [list(ia.ap[0]), [8, NH]]))
                fps2_rep = pp.tile([128, NH], I16)
                fpf = pp.tile([128, NH], F32)
                nc.vector.tensor_copy(fpf[:], fps_rep[:])
                nc.vector.tensor_scalar(
                    fpf[:], fpf[:], boffBf_sb[:, 0:1], None, ALU.add)
                nc.vector.tensor_copy(fps2_rep[:], fpf[:])
                fa = fps_rep[:, :]
                nc.sync.dma_start(
                    fps_dram[:, :], _ap(fa, 0, [[16 * fa.ap[0][0], B], [1, NH]]))
                fb = fps2_rep[:, :]
                nc.sync.dma_start(
                    fps2_dram[:, :], _ap(fb, 0, [[16 * fb.ap[0][0], B], [1, NH]]))

                stkidx = pp.tile([128, 2], I16)
                spidx = pp.tile([128, 16], I16)
                for gi in range(8):
                    nc.sync.dma_start(
                        stkidx[16 * gi:16 * (gi + 1), :],
                        bass.AP(fps_dram[:, :].tensor, fps_dram[:, :].offset + gi * NH,
                                [[1, 16], [16, 2]]))
                    nc.sync.dma_start(
                        spidx[16 * gi:16 * (gi + 1), :],
                        bass.AP(fps2_dram[:, :].tensor, fps2_dram[:, :].offset,
                                [[1, 16], [16, 16]]))

            # ---- stroke-coord + sparse gathers (post-loop) ----
            with tc.tile_pool(name="post", bufs=1) as postp:
                stkrep = postp.tile([128, NSTK, CO], F32)
                sa = stkrep[:, :, :]
                for r in range(16):
                    nc.sync.dma_start(
                        _ap(sa, r * sa.ap[0][0], [[16 * sa.ap[0][0], B], [1, NSTK * CO]]),
                        bass.AP(stk_d, 0, [[NSTK * CO, B], [1, NSTK * CO]]))
                stko = postp.tile([128, NH, CO], F32)
                nc.gpsimd.ap_gather(stko[:], stkrep[:], stkidx[:], channels=128,
                                    num_elems=NSTK, d=CO, num_idxs=NH)
                ka = stko[:, :, :]
                nc.sync.dma_start(
                    bass.AP(stk_o, 0, [[NH * CO, B], [1, NH * CO]]),
                    _ap(ka, 0, [[16 * ka.ap[0][0], B], [1, NH * CO]]))

                spall = postp.tile([128, 2, B * NSTK], F32)
                for h in range(2):
                    for b in range(B):
                        nc.sync.dma_start(
                            spall[:, h, b * NSTK:(b + 1) * NSTK],
                            sparse_d[b, h * 128:(h + 1) * 128, :])
                for h in range(2):
                    spo = wp.tile([128, B * NH, 1], F32, tag="spo")
                    nc.gpsimd.ap_gather(
                        spo[:], spall[:, h, :].rearrange("p (bs u) -> p bs u", u=1),
                        spidx[:], channels=128, num_elems=B * NSTK, d=1, num_idxs=B * NH)
                    nc.sync.dma_start(
                        bass.AP(sparse_o, h * 128 * NH,
                                [[NH, 128], [SP * NH, B], [1, NH]]),
                        spo[:, :, 0])

            # ---- readback prefetch + BN scale/shift + fused GELU + output ----
            with tc.tile_pool(name="yfin", bufs=1) as yfp, \
                 tc.tile_pool(name="ych", bufs=16) as ycp:
                yfin = yfp.tile([128, 2, B * NH * T], F32)
                ychs = []
                for h in range(2):
                    for g in range(16):
                        ych = ycp.tile([128, 512], F32, tag="ych")
                        eng = nc.sync if (h * 16 + g) % 2 == 0 else nc.scalar
                        eng.dma_start(
                            ych[:], y_hbm[h * 128:(h + 1) * 128, g * 512:(g + 1) * 512])
                        ychs.append(ych)

                scale = pp.tile([128, 2], F32)
                shift = pp.tile([128, 2], F32)
                mwork = pp.tile([128, 8], F32)
                for h in range(2):
                    mean = mwork[:, 4 * h:4 * h + 1]
                    var = mwork[:, 4 * h + 1:4 * h + 2]
                    std = mwork[:, 4 * h + 2:4 * h + 3]
                    msq = mwork[:, 4 * h + 3:4 * h + 4]
                    nc.vector.tensor_scalar_mul(mean, gstats[:, h:h + 1], 1.0 / M_GLOBAL)
                    nc.vector.tensor_scalar_mul(var, gstats[:, 2 + h:3 + h], 1.0 / M_GLOBAL)
                    nc.vector.tensor_tensor(msq, mean, mean, ALU.mult)
                    nc.vector.tensor_tensor(var, var, msq, ALU.subtract)
                    nc.vector.tensor_scalar_add(var, var, BN_EPS)
                    nc.scalar.sqrt(std, var)
                    nc.vector.reciprocal(std, std)
                    nc.vector.tensor_tensor(scale[:, h:h + 1], std, gam_sb[:, h:h + 1], ALU.mult)
                    nc.vector.scalar_tensor_tensor(
                        shift[:, h:h + 1], mean, -1.0, scale[:, h:h + 1], ALU.mult, ALU.mult)
                    nc.vector.tensor_tensor(
                        shift[:, h:h + 1], shift[:, h:h + 1], bet_sb[:, h:h + 1], ALU.add)

                for h in range(2):
                    for g in range(16):
                        ych = ychs[h * 16 + g]
                        yh = yfin[:, h, :]
                        dst = _ap(yh, g * 2 * T,
                                  [list(yh.ap[0]), [NH * T, B], [T, 2], [1, T]])
                        nc.scalar.activation(
                            dst, ych[:], ACTF.Gelu_apprx_tanh,
                            bias=shift[:, h:h + 1], scale=scale[:, h:h + 1])
                for h in range(2):
                    for b in range(B):
                        hw = NH * T // 2
                        nc.sync.dma_start(
                            dense_o[b, h * 128:(h + 1) * 128, :hw],
                            yfin[:, h, b * NH * T:b * NH * T + hw])
                        nc.scalar.dma_start(
                            dense_o[b, h * 128:(h + 1) * 128, hw:],
                            yfin[:, h, b * NH * T + hw:(b + 1) * NH * T])

    nc.finalize()
    return nc


def kernel(sparse_fea, dense_fea, stk_coor, conv_w, conv_b, bn_gamma, bn_beta):
    if "nc" not in _CACHE:
        _CACHE["nc"] = _build()
    nc = _CACHE["nc"]

    sparse_fea = np.asarray(sparse_fea, np.float32)
    dense_fea = np.asarray(dense_fea, np.float32)
    stk_coor = np.asarray(stk_coor, np.float32)
    conv_w = np.asarray(conv_w, np.float32)
    conv_b = np.asarray(conv_b, np.float32)
    bn_gamma = np.asarray(bn_gamma, np.float32)
    bn_beta = np.asarray(bn_beta, np.float32)

    in_maps = []
    for c in range(8):
        s = slice(c * B, (c + 1) * B)
        in_maps.append({
            "sparse_fea": sparse_fea[s], "dense_fea": dense_fea[s],
            "stk_coor": stk_coor[s], "conv_w": conv_w, "conv_b": conv_b,
            "bn_gamma": bn_gamma, "bn_beta": bn_beta,
        })
    res = run_bass_kernel_spmd(nc, in_maps, core_ids=list(range(8)),
                               **_CACHE.get("runkw", {}))
    _CACHE["last_result"] = res
    sp = np.concatenate([r["sparse_out"] for r in res.results], 0)
    dn = np.concatenate([r["dense_out"] for r in res.results], 0)
    st = np.concatenate([r["stk_out"] for r in res.results], 0)
    return sp, dn, st


# revision 27
# speedup vs baseline: 1.0071x; 1.0071x over previous
"""Trainium2 Bass kernel for nn_DownSample (FPS + gathers + strided conv + BN + GELU).

Data-parallel over batch: 64 global batches -> 8 cores x 8 batches.
Self-contained: hardcodes all shapes; builds + compiles + runs via
run_bass_kernel_spmd on cores 0-7; returns full (unsharded) outputs.

Per-core plan:
  - FPS over stroke coords via a precomputed pairwise half-distance matrix
    T[n,n'] = 0.5|x_n|^2 + 0.5|x_n'|^2 - <x_n,x_n'> (d = 2*T), replicated
    across 16-partition groups (batch g on partitions 16g..16g+15) so the
    per-iteration distance-column gather is one gpsimd ap_gather (rows of a
    [128 x 32] half-row view; ids 2j+parity).
  - argmax per batch via DVE max/max_index (exact first-occurrence).
  - After every second FPS iteration, the two new strokes' dense data is
    gathered for all batches in one cross-batch ap_gather, rounded to
    float32r into zero-padded 66-wide blocks, and the strided conv for
    those two output strokes runs on the PE at 1 cycle/row, pipelined
    behind the FPS chain. Index vectors are built with tiny bf16 matmuls.
  - Conv outputs (pre-BN, +bias) stream to an HBM scratch; per-channel
    sum/sumsq accumulate on the fly; a 2KB AllReduce produces the global
    batch stats; y is read back and BN+GELU is applied in one ACT pass
    (scale/bias fused into Gelu) while un-permuting, then written out.
"""
import sys

sys.path.insert(0, "/opt/trn_rl_repo")

import numpy as np

import concourse.bacc as bacc
import concourse.mybir as mybir
import concourse.tile as tile
from concourse import tile_rust
from concourse import bass
from concourse.bass_utils import run_bass_kernel_spmd

F32 = mybir.dt.float32
F32R = mybir.dt.float32r
BF16 = mybir.dt.bfloat16
I16 = mybir.dt.int16
U16 = mybir.dt.uint16
ALU = mybir.AluOpType
ACTF = mybir.ActivationFunctionType
AXL = mybir.AxisListType

B = 8          # batches per core
NSTK = 64      # strokes
NPNT = 64      # points per stroke
CO = 32        # coordinate embedding
NH = 32        # sampled strokes
CIN = 128
COUT = 256
SP = 256
T = 32         # conv output positions per stroke
PB = NPNT + 2  # padded conv input block (zero, x0..x63, zero)
BN_EPS = 1e-5
M_GLOBAL = 64 * NH * T  # 65536

_CACHE = {}


def _ap(x, off_delta, dims):
    return bass.AP(x.tensor, x.offset + off_delta, dims)


def _build():
    nc = bacc.Bacc(None, target_bir_lowering=False)

    sparse_d = nc.dram_tensor("sparse_fea", [B, SP, NSTK], F32, kind="ExternalInput")
    dense_d = nc.dram_tensor("dense_fea", [B, CIN, NSTK * NPNT], F32, kind="ExternalInput")
    stk_d = nc.dram_tensor("stk_coor", [B, NSTK, CO], F32, kind="ExternalInput")
    convw_d = nc.dram_tensor("conv_w", [COUT, CIN, 1, 3], F32, kind="ExternalInput")
    convb_d = nc.dram_tensor("conv_b", [COUT], F32, kind="ExternalInput")
    gam_d = nc.dram_tensor("bn_gamma", [COUT], F32, kind="ExternalInput")
    bet_d = nc.dram_tensor("bn_beta", [COUT], F32, kind="ExternalInput")

    sparse_o = nc.dram_tensor("sparse_out", [B, SP, NH], F32, kind="ExternalOutput")
    dense_o = nc.dram_tensor("dense_out", [B, COUT, NH * T], F32, kind="ExternalOutput")
    stk_o = nc.dram_tensor("stk_out", [B, NH, CO], F32, kind="ExternalOutput")

    ident_c = nc.inline_tensor(np.eye(128, dtype=np.float32), "ident128")
    permrow_c = nc.inline_tensor(
        (64.0 * (np.arange(128) % 8)).astype(np.float32).reshape(1, 128), "permrow")
    ones_c = nc.inline_tensor(np.ones((1, 1), np.float32), "ones11")
    boffBf_c = nc.inline_tensor(
        (64.0 * (np.arange(128) // 16)).astype(np.float32).reshape(128, 1), "boffBf")
    parity_c = nc.inline_tensor(
        (np.arange(128) % 2).astype(np.float32).reshape(128, 1), "parity")
    # rank-pair selectors: out[0,n] += in[16*(n%8), r] for rows with (n%16)//8==r
    _n = np.arange(128)
    _P0 = np.zeros((128, 128), np.float32)
    _P1 = np.zeros((128, 128), np.float32)
    _m0 = (_n % 16) // 8 == 0
    _P0[16 * (_n[_m0] % 8), _n[_m0]] = 1.0
    _P1[16 * (_n[~_m0] % 8), _n[~_m0]] = 1.0
    psel0_c = nc.inline_tensor(_P0, "psel0")
    psel1_c = nc.inline_tensor(_P1, "psel1")

    GXW = NSTK * NSTK + NSTK

    with tile.TileContext(nc) as tc:
        with tc.tile_pool(name="dram", bufs=1, space="DRAM") as dpool, \
             tc.tile_pool(name="persist", bufs=1) as pp, \
             tc.tile_pool(name="work", bufs=2) as wp:

            g_hbm = dpool.tile([B, GXW], F32)
            fps_dram = dpool.tile([B, NH], I16)
            fps2_dram = dpool.tile([B, NH], I16)
            y_hbm = dpool.tile([256, 16 * 512], F32)
            cc_in = dpool.tile([128, 4], F32)
            cc_out = dpool.tile([128, 4], F32)

            ident_sb = pp.tile([128, 128], F32)
            nc.sync.dma_start(ident_sb[:], ident_c[:, :])
            ones_sb = pp.tile([1, 1], F32)
            nc.sync.dma_start(ones_sb[:], ones_c[:, :])
            boffBf_sb = pp.tile([128, 1], F32)
            nc.sync.dma_start(boffBf_sb[:], boffBf_c[:, :])
            parity_sb = pp.tile([128, 1], F32)
            nc.sync.dma_start(parity_sb[:], parity_c[:, :])

            # bf16 index-build constants (values exact in bf16)
            psel0_b = pp.tile([128, 128], BF16)
            psel1_b = pp.tile([128, 128], BF16)
            permrow_b = pp.tile([1, 128], BF16)
            ones_b = pp.tile([1, 1], BF16)
            with tc.tile_pool(name="cload", bufs=2) as clp:
                c1 = clp.tile([128, 128], F32, tag="c1")
                nc.sync.dma_start(c1[:], psel0_c[:, :])
                nc.scalar.copy(psel0_b[:], c1[:])
                c2 = clp.tile([128, 128], F32, tag="c1")
                nc.sync.dma_start(c2[:], psel1_c[:, :])
                nc.scalar.copy(psel1_b[:], c2[:])
                c3 = clp.tile([1, 128], F32, tag="c3")
                nc.sync.dma_start(c3[:], permrow_c[:, :])
                nc.scalar.copy(permrow_b[:], c3[:])
                nc.scalar.copy(ones_b[:], ones_sb[:])

            convb_sb = pp.tile([128, 2], F32)
            gam_sb = pp.tile([128, 2], F32)
            bet_sb = pp.tile([128, 2], F32)
            for h in range(2):
                nc.sync.dma_start(convb_sb[:, h:h + 1], convb_d[h * 128:(h + 1) * 128])
                nc.sync.dma_start(gam_sb[:, h:h + 1], gam_d[h * 128:(h + 1) * 128])
                nc.sync.dma_start(bet_sb[:, h:h + 1], bet_d[h * 128:(h + 1) * 128])

            # conv weights -> 6 stationary f32r tiles [i=128, o=128]
            wt_sb = pp.tile([128, 6, 128], F32R)
            with tc.tile_pool(name="psW", bufs=2, space="PSUM") as psW, \
                 tc.tile_pool(name="wload", bufs=1) as wlp:
                wsb = wlp.tile([128, 2, 3 * CIN], F32)
                for h in range(2):
                    nc.sync.dma_start(
                        wsb[:, h, :],
                        convw_d[h * 128:(h + 1) * 128].rearrange("o i u k -> o (i u k)"))
                for h in range(2):
                    for k in range(3):
                        wps = psW.tile([128, 128], F32, tag="wps")
                        w_h = wsb[:, h, :]
                        win = _ap(w_h, k, [list(w_h.ap[0]), [3, CIN]])
                        nc.tensor.transpose(wps[:], win, ident_sb[:])
                        nc.scalar.copy(wt_sb[:, h * 3 + k, :], wps[:])

            # stroke coords: Gram + 0.5*xsq -> g_hbm
            with tc.tile_pool(name="psG", bufs=2, space="PSUM") as psG, \
                 tc.tile_pool(name="gwork", bufs=2) as gw:
                stknat = gw.tile([64, B, CO], F32, tag="stknat")
                for b in range(B):
                    nc.sync.dma_start(stknat[:, b, :], stk_d[b])
                stkT = gw.tile([CO, B * NSTK], F32, tag="stkT")
                for b in range(B):
                    tps = psG.tile([CO, NSTK], F32, tag="tps")
                    nc.tensor.transpose(tps[:], stknat[:, b, :], ident_sb[:64, :64])
                    nc.scalar.copy(stkT[:, b * NSTK:(b + 1) * NSTK], tps[:])
                for b in range(B):
                    gps = psG.tile([NSTK, NSTK], F32, tag="gps")
                    sl = stkT[:, b * NSTK:(b + 1) * NSTK]
                    nc.tensor.matmul(gps[:], sl, sl, start=True, stop=True)
                    gsb = gw.tile([NSTK, NSTK], F32, tag="gsb")
                    nc.scalar.copy(gsb[:], gps[:])
                    nc.sync.dma_start(g_hbm[b:b + 1, :NSTK * NSTK], gsb[:])
                    sq = gw.tile([64, CO], F32, tag="sq")
                    nc.vector.tensor_tensor(sq[:], stknat[:, b, :], stknat[:, b, :], ALU.mult)
                    xs = gw.tile([64, 1], F32, tag="xs")
                    nc.vector.tensor_reduce(xs[:], sq[:], AXL.X, ALU.add)
                    nc.vector.tensor_scalar_mul(xs[:], xs[:], 0.5)
                    nc.sync.dma_start(g_hbm[b:b + 1, NSTK * NSTK:], xs[:])

            # ---------------- big scope: FPS + gathers + conv ----------------
            with tc.tile_pool(name="big", bufs=1) as bigp:
                # replicate G and xsq/2 across 16-partition groups; T in place
                txs = bigp.tile([128, GXW], F32)
                tfull = txs[:, :]
                ta = _ap(tfull, 0, [list(tfull.ap[0]), [NSTK, NSTK], [1, NSTK]])
                xa = _ap(tfull, NSTK * NSTK, [list(tfull.ap[0]), [1, NSTK]])
                last_trep = None
                for r in range(16):
                    last_trep = nc.sync.dma_start(
                        _ap(tfull, r * tfull.ap[0][0],
                            [[16 * tfull.ap[0][0], B], [1, GXW]]),
                        g_hbm[:, :])
                xsq_n = _ap(xa, 0, [list(xa.ap[0]), [1, NSTK], [0, NSTK]])
                xsq_np = _ap(xa, 0, [list(xa.ap[0]), [0, NSTK], [1, NSTK]])
                nc.vector.scalar_tensor_tensor(
                    ta, ta, -1.0, xsq_n, ALU.mult, ALU.add)
                nc.vector.tensor_tensor(ta, ta, xsq_np, ALU.add)
                # half-row view for the FPS column gather
                t_half = _ap(ta, 0, [list(ta.ap[0]), [NSTK // 2, 2 * NSTK],
                                     [1, NSTK // 2]])

                # dense features [i, (b,s), p]; issue after the T-path DMAs
                dall = bigp.tile([128, B * NSTK, NPNT], F32)
                for b in range(B):
                    dv = dense_d[b].rearrange("i (s p) -> i s p", p=NPNT)
                    half = NSTK // 2
                    d1 = nc.sync.dma_start(
                        dall[:, b * NSTK:b * NSTK + half, :], dv[:, :half, :])
                    if b == 0:
                        tile_rust.add_dep_helper(
                            d1.ins, last_trep.ins, sync=False,
                            reason="T replication drains the queue before bulk dense")
                    nc.sync.dma_start(
                        dall[:, b * NSTK + half:(b + 1) * NSTK, :], dv[:, half:, :])

                # FPS state
                dists = pp.tile([128, NSTK], F32)
                nc.vector.tensor_scalar_mul(
                    dists[:], _ap(ta, 0, [list(ta.ap[0]), [NSTK, NSTK]]), 2.0)
                ix8buf = pp.tile([128, 8 * NH], U16)
                nc.vector.memset(ix8buf[:], 0)
                mx8 = pp.tile([128, 8], F32)
                tcol = pp.tile([128, 16, NSTK // 2], F32)
                sums = pp.tile([128, 2, 16], F32)
                sqs = pp.tile([128, 2, 16], F32)
                sqscr = pp.tile([128, 512], F32)

                with tc.tile_pool(name="temp", bufs=2) as tpool, \
                     tc.tile_pool(name="psC", bufs=3, space="PSUM") as psC, \
                     tc.tile_pool(name="psJ", bufs=2, space="PSUM") as psJ, \
                     tc.tile_pool(name="psI", bufs=2, space="PSUM") as psI, \
                     tc.tile_pool(name="ystage", bufs=2) as ysp:
                    for i in range(NH):
                        if i >= 1:
                            nc.vector.max(mx8[:], dists[:])
                            nc.vector.max_index(
                                ix8buf[:, 8 * i:8 * i + 8], mx8[:], dists[:])
                            # half-row ids {2j, 2j+1} by partition parity
                            tji = wp.tile([128, 1], I16, tag="tji")
                            nc.vector.scalar_tensor_tensor(
                                tji[:], ix8buf[:, 8 * i:8 * i + 1], 2.0,
                                parity_sb[:], ALU.mult, ALU.add)
                            nc.gpsimd.ap_gather(
                                tcol[:], t_half, tji[:], channels=128,
                                num_elems=2 * NSTK, d=NSTK // 2, num_idxs=16)
                            nc.vector.scalar_tensor_tensor(
                                dists[:],
                                _ap(tcol[:, :, :], 0, [list(tcol[:, :, :].ap[0]),
                                                       [1, NSTK]]),
                                2.0, dists[:], ALU.mult, ALU.min)

                        if i % 2 == 1:
                            g = i // 2
                            # pair ids: idx[p] = 64*(p%8) + j^{(p%16)//8}_{p%8}
                            jf2 = wp.tile([128, 2], BF16, tag="jf2")
                            ib = ix8buf[:, :]
                            nc.vector.tensor_copy(
                                jf2[:], _ap(ib, 8 * (i - 1), [list(ib.ap[0]), [8, 2]]))
                            jrow_ps = psJ.tile([1, 128], F32, tag="jrow")
                            nc.tensor.matmul(jrow_ps[:], jf2[:, 0:1], psel0_b[:],
                                             start=True, stop=False)
                            nc.tensor.matmul(jrow_ps[:], jf2[:, 1:2], psel1_b[:],
                                             start=False, stop=True)
                            jrow_sb = wp.tile([1, 128], BF16, tag="jrowsb")
                            nc.scalar.copy(jrow_sb[:], jrow_ps[:])
                            idx_ps = psI.tile([128, 1], F32, tag="idxps")
                            nc.tensor.matmul(
                                idx_ps[:], jrow_sb[:], ones_b[:], start=True, stop=False)
                            nc.tensor.matmul(
                                idx_ps[:], permrow_b[:], ones_b[:], start=False, stop=True)
                            idxd = wp.tile([128, 1], I16, tag="idxd")
                            nc.vector.tensor_copy(idxd[:], idx_ps[:])

                            temp = tpool.tile([128, 16, NPNT], F32, tag="temp")
                            nc.gpsimd.ap_gather(
                                temp[:], dall[:], idxd[:],
                                channels=128, num_elems=B * NSTK, d=NPNT, num_idxs=16)
                            # round into zero-padded f32r blocks [z, x0..x63, z]
                            tempr = tpool.tile([128, 16, PB], F32R, tag="tempr")
                            tr = tempr[:, :, :]
                            nc.vector.memset(
                                _ap(tr, 0, [list(tr.ap[0]), [PB, 16],
                                            [PB - 1, 2]]).bitcast(F32), 0)
                            nc.scalar.copy(
                                _ap(tr, 1, [list(tr.ap[0]), [PB, 16], [1, NPNT]]),
                                temp[:])
                            for h in range(2):
                                cps = psC.tile([128, 512], F32, tag="cps")
                                ca = cps[:, :]
                                for k in range(3):
                                    nc.tensor.matmul(
                                        _ap(ca, 0, [list(ca.ap[0]), [2 * T, B],
                                                    [T, 2], [1, T]]),
                                        wt_sb[:, h * 3 + k, :],
                                        _ap(tr, k, [list(tr.ap[0]), [PB, B],
                                                    [8 * PB, 2], [2, T]]),
                                        start=(k == 0), stop=(k == 2),
                                        skip_group_check=(k > 0))
                                yst = ysp.tile([128, 512], F32, tag="yst")
                                nc.scalar.activation(
                                    yst[:], cps[:], ACTF.Identity,
                                    bias=convb_sb[:, h:h + 1], scale=1.0,
                                    accum_out=sums[:, h, g:g + 1])
                                nc.vector.scalar_tensor_tensor(
                                    sqscr[:], yst[:], 1.0, yst[:], ALU.mult, ALU.mult,
                                    accum_out=sqs[:, h, g:g + 1])
                                nc.sync.dma_start(
                                    y_hbm[h * 128:(h + 1) * 128, g * 512:(g + 1) * 512],
                                    yst[:])

                # ---- BN stats + allreduce ----
                stats = pp.tile([128, 4], F32)
                for h in range(2):
                    nc.vector.tensor_reduce(stats[:, h:h + 1], sums[:, h, :], AXL.X, ALU.add)
                    nc.vector.tensor_reduce(stats[:, 2 + h:3 + h], sqs[:, h, :], AXL.X, ALU.add)
                nc.gpsimd.dma_start(cc_in[:, :], stats[:])
                nc.gpsimd.collective_compute(
                    "AllReduce", ALU.add, replica_groups=[list(range(8))],
                    ins=[cc_in[:, :]], outs=[cc_out[:, :]])
                gstats = pp.tile([128, 4], F32)
                nc.gpsimd.dma_start(gstats[:], cc_out[:, :])

                # ---- post-loop index tables ----
                fps_rep = pp.tile([128, NH], I16)
                ia = ix8buf[:, :]
                nc.vector.tensor_copy(fps_rep[:], _ap(ia, 0, [list(ia.ap[0]), [8, NH]]))
                fps2_rep = pp.tile([128, NH], I16)
                fpf = pp.tile([128, NH], F32)
                nc.vector.tensor_copy(fpf[:], fps_rep[:])
                nc.vector.tensor_scalar(
                    fpf[:], fpf[:], boffBf_sb[:, 0:1], None, ALU.add)
                nc.vector.tensor_copy(fps2_rep[:], fpf[:])
                fa = fps_rep[:, :]
                nc.sync.dma_start(
                    fps_dram[:, :], _ap(fa, 0, [[16 * fa.ap[0][0], B], [1, NH]]))
                fb = fps2_rep[:, :]
                nc.sync.dma_start(
                    fps2_dram[:, :], _ap(fb, 0, [[16 * fb.ap[0][0], B], [1, NH]]))

                stkidx = pp.tile([128, 2], I16)
                spidx = pp.tile([128, 16], I16)
                for gi in range(8):
                    nc.sync.dma_start(
                        stkidx[16 * gi:16 * (gi + 1), :],
                        bass.AP(fps_dram[:, :].tensor, fps_dram[:, :].offset + gi * NH,
                                [[1, 16], [16, 2]]))
                    nc.sync.dma_start(
                        spidx[16 * gi:16 * (gi + 1), :],
                        bass.AP(fps2_dram[:, :].tensor, fps2_dram[:, :].offset,
                                [[1, 16], [16, 16]]))

            # ---- stroke-coord + sparse gathers (post-loop) ----
            with tc.tile_pool(name="post", bufs=1) as postp:
                stkrep = postp.tile([128, NSTK, CO], F32)
                sa = stkrep[:, :, :]
                for r in range(16):
                    nc.sync.dma_start(
                        _ap(sa, r * sa.ap[0][0], [[16 * sa.ap[0][0], B], [1, NSTK * CO]]),
                        bass.AP(stk_d, 0, [[NSTK * CO, B], [1, NSTK * CO]]))
                stko = postp.tile([128, NH, CO], F32)
                nc.gpsimd.ap_gather(stko[:], stkrep[:], stkidx[:], channels=128,
                                    num_elems=NSTK, d=CO, num_idxs=NH)
                ka = stko[:, :, :]
                nc.sync.dma_start(
                    bass.AP(stk_o, 0, [[NH * CO, B], [1, NH * CO]]),
                    _ap(ka, 0, [[16 * ka.ap[0][0], B], [1, NH * CO]]))

                spall = postp.tile([128, 2, B * NSTK], F32)
                for h in range(2):
                    for b in range(B):
                        nc.sync.dma_start(
                            spall[:, h, b * NSTK:(b + 1) * NSTK],
                            sparse_d[b, h * 128:(h + 1) * 128, :])
                for h in range(2):
                    spo = wp.tile([128, B * NH, 1], F32, tag="spo")
                    nc.gpsimd.ap_gather(
                        spo[:], spall[:, h, :].rearrange("p (bs u) -> p bs u", u=1),
                        spidx[:], channels=128, num_elems=B * NSTK, d=1, num_idxs=B * NH)
                    nc.sync.dma_start(
                        bass.AP(sparse_o, h * 128 * NH,
                                [[NH, 128], [SP * NH, B], [1, NH]]),
                        spo[:, :, 0])

            # ---- readback prefetch + BN scale/shift + fused GELU + output ----
            with tc.tile_pool(name="yfin", bufs=1) as yfp, \
                 tc.tile_pool(name="ych", bufs=16) as ycp:
                yfin = yfp.tile([128, 2, B * NH * T], F32)
                ychs = []
                for h in range(2):
                    for g in range(16):
                        ych = ycp.tile([128, 512], F32, tag="ych")
                        nc.sync.dma_start(
                            ych[:], y_hbm[h * 128:(h + 1) * 128, g * 512:(g + 1) * 512])
                        ychs.append(ych)

                scale = pp.tile([128, 2], F32)
                shift = pp.tile([128, 2], F32)
                mwork = pp.tile([128, 8], F32)
                for h in range(2):
                    mean = mwork[:, 4 * h:4 * h + 1]
                    var = mwork[:, 4 * h + 1:4 * h + 2]
                    std = mwork[:, 4 * h + 2:4 * h + 3]
                    msq = mwork[:, 4 * h + 3:4 * h + 4]
                    nc.vector.tensor_scalar_mul(mean, gstats[:, h:h + 1], 1.0 / M_GLOBAL)
                    nc.vector.tensor_scalar_mul(var, gstats[:, 2 + h:3 + h], 1.0 / M_GLOBAL)
                    nc.vector.tensor_tensor(msq, mean, mean, ALU.mult)
                    nc.vector.tensor_tensor(var, var, msq, ALU.subtract)
                    nc.vector.tensor_scalar_add(var, var, BN_EPS)
                    nc.scalar.sqrt(std, var)
                    nc.vector.reciprocal(std, std)
                    nc.vector.tensor_tensor(scale[:, h:h + 1], std, gam_sb[:, h:h + 1], ALU.mult)
                    nc.vector.scalar_tensor_tensor(
                        shift[:, h:h + 1], mean, -1.0, scale[:, h:h + 1], ALU.mult, ALU.mult)
                    nc.vector.tensor_tensor(
                        shift[:, h:h + 1], shift[:, h:h + 1], bet_sb[:, h:h + 1], ALU.add)

                for h in range(2):
                    for g in range(16):
                        ych = ychs[h * 16 + g]
                        yh = yfin[:, h, :]
                        dst = _ap(yh, g * 2 * T,
                                  [list(yh.ap[0]), [NH * T, B], [T, 2], [1, T]])
                        nc.scalar.activation(
                            dst, ych[:], ACTF.Gelu_apprx_tanh,
                            bias=shift[:, h:h + 1], scale=scale[:, h:h + 1])
                for h in range(2):
                    for b in range(B):
                        hw = NH * T // 2
                        nc.sync.dma_start(
                            dense_o[b, h * 128:(h + 1) * 128, :hw],
                            yfin[:, h, b * NH * T:b * NH * T + hw])
                        nc.sync.dma_start(
                            dense_o[b, h * 128:(h + 1) * 128, hw:],
                            yfin[:, h, b * NH * T + hw:(b + 1) * NH * T])

    nc.finalize()
    return nc


def kernel(sparse_fea, dense_fea, stk_coor, conv_w, conv_b, bn_gamma, bn_beta):
    if "nc" not in _CACHE:
        _CACHE["nc"] = _build()
    nc = _CACHE["nc"]

    sparse_fea = np.asarray(sparse_fea, np.float32)
    dense_fea = np.asarray(dense_fea, np.float32)
    stk_coor = np.asarray(stk_coor, np.float32)
    conv_w = np.asarray(conv_w, np.float32)
    conv_b = np.asarray(conv_b, np.float32)
    bn_gamma = np.asarray(bn_gamma, np.float32)
    bn_beta = np.asarray(bn_beta, np.float32)

    in_maps = []
    for c in range(8):
        s = slice(c * B, (c + 1) * B)
        in_maps.append({
            "sparse_fea": sparse_fea[s], "dense_fea": dense_fea[s],
            "stk_coor": stk_coor[s], "conv_w": conv_w, "conv_b": conv_b,
            "bn_gamma": bn_gamma, "bn_beta": bn_beta,
        })
    res = run_bass_kernel_spmd(nc, in_maps, core_ids=list(range(8)),
                               **_CACHE.get("runkw", {}))
    _CACHE["last_result"] = res
    sp = np.concatenate([r["sparse_out"] for r in res.results], 0)
    dn = np.concatenate([r["dense_out"] for r in res.results], 0)
    st = np.concatenate([r["stk_out"] for r in res.results], 0)
    return sp, dn, st


# revision 28
# speedup vs baseline: 1.0826x; 1.0749x over previous
"""Trainium2 Bass kernel for nn_DownSample (FPS + gathers + strided conv + BN + GELU).

Data-parallel over batch: 64 global batches -> 8 cores x 8 batches.
Self-contained: hardcodes all shapes; builds + compiles + runs via
run_bass_kernel_spmd on cores 0-7; returns full (unsharded) outputs.

Per-core plan:
  - FPS over stroke coords via a precomputed pairwise half-distance matrix
    T[n,n'] = 0.5|x_n|^2 + 0.5|x_n'|^2 - <x_n,x_n'> (d = 2*T), replicated
    across 16-partition groups (batch g on partitions 16g..16g+15) so the
    per-iteration distance-column gather is one gpsimd ap_gather (rows of a
    [128 x 32] half-row view; ids 2j+parity).
  - argmax per batch via DVE max/max_index (exact first-occurrence).
  - After every second FPS iteration, the two new strokes' dense data is
    gathered for all batches in one cross-batch ap_gather, rounded to
    float32r into zero-padded 66-wide blocks, and the strided conv for
    those two output strokes runs on the PE at 1 cycle/row, pipelined
    behind the FPS chain. Index vectors are built with tiny bf16 matmuls.
  - Conv outputs (pre-BN, +bias) stream to an HBM scratch; per-channel
    sum/sumsq accumulate on the fly; a 2KB AllReduce produces the global
    batch stats; y is read back and BN+GELU is applied in one ACT pass
    (scale/bias fused into Gelu) while un-permuting, then written out.
"""
import sys

sys.path.insert(0, "/opt/trn_rl_repo")

import numpy as np

import concourse.bacc as bacc
import concourse.mybir as mybir
import concourse.tile as tile
from concourse import tile_rust
from concourse import bass
from concourse.bass_utils import run_bass_kernel_spmd

F32 = mybir.dt.float32
F32R = mybir.dt.float32r
BF16 = mybir.dt.bfloat16
I16 = mybir.dt.int16
U16 = mybir.dt.uint16
ALU = mybir.AluOpType
ACTF = mybir.ActivationFunctionType
AXL = mybir.AxisListType

B = 8          # batches per core
NSTK = 64      # strokes
NPNT = 64      # points per stroke
CO = 32        # coordinate embedding
NH = 32        # sampled strokes
CIN = 128
COUT = 256
SP = 256
T = 32         # conv output positions per stroke
PB = NPNT + 2  # padded conv input block (zero, x0..x63, zero)
BN_EPS = 1e-5
M_GLOBAL = 64 * NH * T  # 65536

_CACHE = {}


def _ap(x, off_delta, dims):
    return bass.AP(x.tensor, x.offset + off_delta, dims)


def _build():
    nc = bacc.Bacc(None, target_bir_lowering=False)

    sparse_d = nc.dram_tensor("sparse_fea", [B, SP, NSTK], F32, kind="ExternalInput")
    dense_d = nc.dram_tensor("dense_fea", [B, CIN, NSTK * NPNT], F32, kind="ExternalInput")
    stk_d = nc.dram_tensor("stk_coor", [B, NSTK, CO], F32, kind="ExternalInput")
    convw_d = nc.dram_tensor("conv_w", [COUT, CIN, 1, 3], F32, kind="ExternalInput")
    convb_d = nc.dram_tensor("conv_b", [COUT], F32, kind="ExternalInput")
    gam_d = nc.dram_tensor("bn_gamma", [COUT], F32, kind="ExternalInput")
    bet_d = nc.dram_tensor("bn_beta", [COUT], F32, kind="ExternalInput")

    sparse_o = nc.dram_tensor("sparse_out", [B, SP, NH], F32, kind="ExternalOutput")
    dense_o = nc.dram_tensor("dense_out", [B, COUT, NH * T], F32, kind="ExternalOutput")
    stk_o = nc.dram_tensor("stk_out", [B, NH, CO], F32, kind="ExternalOutput")

    ident_c = nc.inline_tensor(np.eye(128, dtype=np.float32), "ident128")
    permrow_c = nc.inline_tensor(
        (64.0 * (np.arange(128) % 8)).astype(np.float32).reshape(1, 128), "permrow")
    ones_c = nc.inline_tensor(np.ones((1, 1), np.float32), "ones11")
    boffBf_c = nc.inline_tensor(
        (64.0 * (np.arange(128) // 16)).astype(np.float32).reshape(128, 1), "boffBf")
    parity_c = nc.inline_tensor(
        (np.arange(128) % 2).astype(np.float32).reshape(128, 1), "parity")
    # rank-pair selectors: out[0,n] += in[16*(n%8), r] for rows with (n%16)//8==r
    _n = np.arange(128)
    _P0 = np.zeros((128, 128), np.float32)
    _P1 = np.zeros((128, 128), np.float32)
    _m0 = (_n % 16) // 8 == 0
    _P0[16 * (_n[_m0] % 8), _n[_m0]] = 1.0
    _P1[16 * (_n[~_m0] % 8), _n[~_m0]] = 1.0
    psel0_c = nc.inline_tensor(_P0, "psel0")
    psel1_c = nc.inline_tensor(_P1, "psel1")

    GXW = NSTK * NSTK + NSTK

    with tile.TileContext(nc) as tc:
        with tc.tile_pool(name="dram", bufs=1, space="DRAM") as dpool, \
             tc.tile_pool(name="persist", bufs=1) as pp, \
             tc.tile_pool(name="work", bufs=2) as wp:

            g_hbm = dpool.tile([B, GXW], F32)
            fps_dram = dpool.tile([B, NH], I16)
            fps2_dram = dpool.tile([B, NH], I16)
            y_hbm = dpool.tile([256, 16 * 512], BF16)
            cc_in = dpool.tile([128, 4], F32)
            cc_out = dpool.tile([128, 4], F32)

            ident_sb = pp.tile([128, 128], F32)
            nc.sync.dma_start(ident_sb[:], ident_c[:, :])
            ones_sb = pp.tile([1, 1], F32)
            nc.sync.dma_start(ones_sb[:], ones_c[:, :])
            boffBf_sb = pp.tile([128, 1], F32)
            nc.sync.dma_start(boffBf_sb[:], boffBf_c[:, :])
            parity_sb = pp.tile([128, 1], F32)
            nc.sync.dma_start(parity_sb[:], parity_c[:, :])

            # bf16 index-build constants (values exact in bf16)
            psel0_b = pp.tile([128, 128], BF16)
            psel1_b = pp.tile([128, 128], BF16)
            permrow_b = pp.tile([1, 128], BF16)
            ones_b = pp.tile([1, 1], BF16)
            with tc.tile_pool(name="cload", bufs=2) as clp:
                c1 = clp.tile([128, 128], F32, tag="c1")
                nc.sync.dma_start(c1[:], psel0_c[:, :])
                nc.scalar.copy(psel0_b[:], c1[:])
                c2 = clp.tile([128, 128], F32, tag="c1")
                nc.sync.dma_start(c2[:], psel1_c[:, :])
                nc.scalar.copy(psel1_b[:], c2[:])
                c3 = clp.tile([1, 128], F32, tag="c3")
                nc.sync.dma_start(c3[:], permrow_c[:, :])
                nc.scalar.copy(permrow_b[:], c3[:])
                nc.scalar.copy(ones_b[:], ones_sb[:])

            convb_sb = pp.tile([128, 2], F32)
            gam_sb = pp.tile([128, 2], F32)
            bet_sb = pp.tile([128, 2], F32)
            for h in range(2):
                nc.sync.dma_start(convb_sb[:, h:h + 1], convb_d[h * 128:(h + 1) * 128])
                nc.sync.dma_start(gam_sb[:, h:h + 1], gam_d[h * 128:(h + 1) * 128])
                nc.sync.dma_start(bet_sb[:, h:h + 1], bet_d[h * 128:(h + 1) * 128])

            # conv weights -> 6 stationary f32r tiles [i=128, o=128]
            wt_sb = pp.tile([128, 6, 128], F32R)
            with tc.tile_pool(name="psW", bufs=2, space="PSUM") as psW, \
                 tc.tile_pool(name="wload", bufs=1) as wlp:
                wsb = wlp.tile([128, 2, 3 * CIN], F32)
                for h in range(2):
                    nc.sync.dma_start(
                        wsb[:, h, :],
                        convw_d[h * 128:(h + 1) * 128].rearrange("o i u k -> o (i u k)"))
                for h in range(2):
                    for k in range(3):
                        wps = psW.tile([128, 128], F32, tag="wps")
                        w_h = wsb[:, h, :]
                        win = _ap(w_h, k, [list(w_h.ap[0]), [3, CIN]])
                        nc.tensor.transpose(wps[:], win, ident_sb[:])
                        nc.scalar.copy(wt_sb[:, h * 3 + k, :], wps[:])

            # stroke coords: Gram + 0.5*xsq -> g_hbm
            with tc.tile_pool(name="psG", bufs=2, space="PSUM") as psG, \
                 tc.tile_pool(name="gwork", bufs=2) as gw:
                stknat = gw.tile([64, B, CO], F32, tag="stknat")
                for b in range(B):
                    nc.sync.dma_start(stknat[:, b, :], stk_d[b])
                stkT = gw.tile([CO, B * NSTK], F32, tag="stkT")
                for b in range(B):
                    tps = psG.tile([CO, NSTK], F32, tag="tps")
                    nc.tensor.transpose(tps[:], stknat[:, b, :], ident_sb[:64, :64])
                    nc.scalar.copy(stkT[:, b * NSTK:(b + 1) * NSTK], tps[:])
                for b in range(B):
                    gps = psG.tile([NSTK, NSTK], F32, tag="gps")
                    sl = stkT[:, b * NSTK:(b + 1) * NSTK]
                    nc.tensor.matmul(gps[:], sl, sl, start=True, stop=True)
                    gsb = gw.tile([NSTK, NSTK], F32, tag="gsb")
                    nc.scalar.copy(gsb[:], gps[:])
                    nc.sync.dma_start(g_hbm[b:b + 1, :NSTK * NSTK], gsb[:])
                    sq = gw.tile([64, CO], F32, tag="sq")
                    nc.vector.tensor_tensor(sq[:], stknat[:, b, :], stknat[:, b, :], ALU.mult)
                    xs = gw.tile([64, 1], F32, tag="xs")
                    nc.vector.tensor_reduce(xs[:], sq[:], AXL.X, ALU.add)
                    nc.vector.tensor_scalar_mul(xs[:], xs[:], 0.5)
                    nc.sync.dma_start(g_hbm[b:b + 1, NSTK * NSTK:], xs[:])

            # ---------------- big scope: FPS + gathers + conv ----------------
            with tc.tile_pool(name="big", bufs=1) as bigp:
                # replicate G and xsq/2 across 16-partition groups; T in place
                txs = bigp.tile([128, GXW], F32)
                tfull = txs[:, :]
                ta = _ap(tfull, 0, [list(tfull.ap[0]), [NSTK, NSTK], [1, NSTK]])
                xa = _ap(tfull, NSTK * NSTK, [list(tfull.ap[0]), [1, NSTK]])
                last_trep = None
                for r in range(16):
                    last_trep = nc.sync.dma_start(
                        _ap(tfull, r * tfull.ap[0][0],
                            [[16 * tfull.ap[0][0], B], [1, GXW]]),
                        g_hbm[:, :])
                xsq_n = _ap(xa, 0, [list(xa.ap[0]), [1, NSTK], [0, NSTK]])
                xsq_np = _ap(xa, 0, [list(xa.ap[0]), [0, NSTK], [1, NSTK]])
                nc.vector.scalar_tensor_tensor(
                    ta, ta, -1.0, xsq_n, ALU.mult, ALU.add)
                nc.vector.tensor_tensor(ta, ta, xsq_np, ALU.add)
                # half-row view for the FPS column gather
                t_half = _ap(ta, 0, [list(ta.ap[0]), [NSTK // 2, 2 * NSTK],
                                     [1, NSTK // 2]])

                # dense features [i, (b,s), p]; issue after the T-path DMAs
                dall = bigp.tile([128, B * NSTK, NPNT], F32)
                for b in range(B):
                    dv = dense_d[b].rearrange("i (s p) -> i s p", p=NPNT)
                    half = NSTK // 2
                    d1 = nc.sync.dma_start(
                        dall[:, b * NSTK:b * NSTK + half, :], dv[:, :half, :])
                    if b == 0:
                        tile_rust.add_dep_helper(
                            d1.ins, last_trep.ins, sync=False,
                            reason="T replication drains the queue before bulk dense")
                    nc.sync.dma_start(
                        dall[:, b * NSTK + half:(b + 1) * NSTK, :], dv[:, half:, :])

                # FPS state
                dists = pp.tile([128, NSTK], F32)
                nc.vector.tensor_scalar_mul(
                    dists[:], _ap(ta, 0, [list(ta.ap[0]), [NSTK, NSTK]]), 2.0)
                ix8buf = pp.tile([128, 8 * NH], U16)
                nc.vector.memset(ix8buf[:], 0)
                mx8 = pp.tile([128, 8], F32)
                tcol = pp.tile([128, 16, NSTK // 2], F32)
                sums = pp.tile([128, 2, 16], F32)
                sqs = pp.tile([128, 2, 16], F32)
                sqscr = pp.tile([128, 512], F32)

                with tc.tile_pool(name="temp", bufs=2) as tpool, \
                     tc.tile_pool(name="psC", bufs=3, space="PSUM") as psC, \
                     tc.tile_pool(name="psJ", bufs=2, space="PSUM") as psJ, \
                     tc.tile_pool(name="psI", bufs=2, space="PSUM") as psI, \
                     tc.tile_pool(name="ystage", bufs=2) as ysp:
                    for i in range(NH):
                        if i >= 1:
                            nc.vector.max(mx8[:], dists[:])
                            nc.vector.max_index(
                                ix8buf[:, 8 * i:8 * i + 8], mx8[:], dists[:])
                            # half-row ids {2j, 2j+1} by partition parity
                            tji = wp.tile([128, 1], I16, tag="tji")
                            nc.vector.scalar_tensor_tensor(
                                tji[:], ix8buf[:, 8 * i:8 * i + 1], 2.0,
                                parity_sb[:], ALU.mult, ALU.add)
                            nc.gpsimd.ap_gather(
                                tcol[:], t_half, tji[:], channels=128,
                                num_elems=2 * NSTK, d=NSTK // 2, num_idxs=16)
                            nc.vector.scalar_tensor_tensor(
                                dists[:],
                                _ap(tcol[:, :, :], 0, [list(tcol[:, :, :].ap[0]),
                                                       [1, NSTK]]),
                                2.0, dists[:], ALU.mult, ALU.min)

                        if i % 2 == 1:
                            g = i // 2
                            # pair ids: idx[p] = 64*(p%8) + j^{(p%16)//8}_{p%8}
                            jf2 = wp.tile([128, 2], BF16, tag="jf2")
                            ib = ix8buf[:, :]
                            nc.vector.tensor_copy(
                                jf2[:], _ap(ib, 8 * (i - 1), [list(ib.ap[0]), [8, 2]]))
                            jrow_ps = psJ.tile([1, 128], F32, tag="jrow")
                            nc.tensor.matmul(jrow_ps[:], jf2[:, 0:1], psel0_b[:],
                                             start=True, stop=False)
                            nc.tensor.matmul(jrow_ps[:], jf2[:, 1:2], psel1_b[:],
                                             start=False, stop=True)
                            jrow_sb = wp.tile([1, 128], BF16, tag="jrowsb")
                            nc.scalar.copy(jrow_sb[:], jrow_ps[:])
                            idx_ps = psI.tile([128, 1], F32, tag="idxps")
                            nc.tensor.matmul(
                                idx_ps[:], jrow_sb[:], ones_b[:], start=True, stop=False)
                            nc.tensor.matmul(
                                idx_ps[:], permrow_b[:], ones_b[:], start=False, stop=True)
                            idxd = wp.tile([128, 1], I16, tag="idxd")
                            nc.vector.tensor_copy(idxd[:], idx_ps[:])

                            temp = tpool.tile([128, 16, NPNT], F32, tag="temp")
                            nc.gpsimd.ap_gather(
                                temp[:], dall[:], idxd[:],
                                channels=128, num_elems=B * NSTK, d=NPNT, num_idxs=16)
                            # round into zero-padded f32r blocks [z, x0..x63, z]
                            tempr = tpool.tile([128, 16, PB], F32R, tag="tempr")
                            tr = tempr[:, :, :]
                            nc.vector.memset(
                                _ap(tr, 0, [list(tr.ap[0]), [PB, 16],
                                            [PB - 1, 2]]).bitcast(F32), 0)
                            nc.scalar.copy(
                                _ap(tr, 1, [list(tr.ap[0]), [PB, 16], [1, NPNT]]),
                                temp[:])
                            for h in range(2):
                                cps = psC.tile([128, 512], F32, tag="cps")
                                ca = cps[:, :]
                                for k in range(3):
                                    nc.tensor.matmul(
                                        _ap(ca, 0, [list(ca.ap[0]), [2 * T, B],
                                                    [T, 2], [1, T]]),
                                        wt_sb[:, h * 3 + k, :],
                                        _ap(tr, k, [list(tr.ap[0]), [PB, B],
                                                    [8 * PB, 2], [2, T]]),
                                        start=(k == 0), stop=(k == 2),
                                        skip_group_check=(k > 0))
                                yst = ysp.tile([128, 512], BF16, tag="yst")
                                nc.scalar.activation(
                                    yst[:], cps[:], ACTF.Identity,
                                    bias=convb_sb[:, h:h + 1], scale=1.0,
                                    accum_out=sums[:, h, g:g + 1])
                                nc.vector.scalar_tensor_tensor(
                                    sqscr[:], yst[:], 1.0, yst[:], ALU.mult, ALU.mult,
                                    accum_out=sqs[:, h, g:g + 1])
                                nc.sync.dma_start(
                                    y_hbm[h * 128:(h + 1) * 128, g * 512:(g + 1) * 512],
                                    yst[:])

                # ---- BN stats + allreduce ----
                stats = pp.tile([128, 4], F32)
                for h in range(2):
                    nc.vector.tensor_reduce(stats[:, h:h + 1], sums[:, h, :], AXL.X, ALU.add)
                    nc.vector.tensor_reduce(stats[:, 2 + h:3 + h], sqs[:, h, :], AXL.X, ALU.add)
                nc.gpsimd.dma_start(cc_in[:, :], stats[:])
                nc.gpsimd.collective_compute(
                    "AllReduce", ALU.add, replica_groups=[list(range(8))],
                    ins=[cc_in[:, :]], outs=[cc_out[:, :]])
                gstats = pp.tile([128, 4], F32)
                nc.gpsimd.dma_start(gstats[:], cc_out[:, :])

                # ---- post-loop index tables ----
                fps_rep = pp.tile([128, NH], I16)
                ia = ix8buf[:, :]
                nc.vector.tensor_copy(fps_rep[:], _ap(ia, 0, [list(ia.ap[0]), [8, NH]]))
                fps2_rep = pp.tile([128, NH], I16)
                fpf = pp.tile([128, NH], F32)
                nc.vector.tensor_copy(fpf[:], fps_rep[:])
                nc.vector.tensor_scalar(
                    fpf[:], fpf[:], boffBf_sb[:, 0:1], None, ALU.add)
                nc.vector.tensor_copy(fps2_rep[:], fpf[:])
                fa = fps_rep[:, :]
                nc.sync.dma_start(
                    fps_dram[:, :], _ap(fa, 0, [[16 * fa.ap[0][0], B], [1, NH]]))
                fb = fps2_rep[:, :]
                nc.sync.dma_start(
                    fps2_dram[:, :], _ap(fb, 0, [[16 * fb.ap[0][0], B], [1, NH]]))

                stkidx = pp.tile([128, 2], I16)
                spidx = pp.tile([128, 16], I16)
                for gi in range(8):
                    nc.sync.dma_start(
                        stkidx[16 * gi:16 * (gi + 1), :],
                        bass.AP(fps_dram[:, :].tensor, fps_dram[:, :].offset + gi * NH,
                                [[1, 16], [16, 2]]))
                    nc.sync.dma_start(
                        spidx[16 * gi:16 * (gi + 1), :],
                        bass.AP(fps2_dram[:, :].tensor, fps2_dram[:, :].offset,
                                [[1, 16], [16, 16]]))

            # ---- stroke-coord + sparse gathers (post-loop) ----
            with tc.tile_pool(name="post", bufs=1) as postp:
                stkrep = postp.tile([128, NSTK, CO], F32)
                sa = stkrep[:, :, :]
                for r in range(16):
                    nc.sync.dma_start(
                        _ap(sa, r * sa.ap[0][0], [[16 * sa.ap[0][0], B], [1, NSTK * CO]]),
                        bass.AP(stk_d, 0, [[NSTK * CO, B], [1, NSTK * CO]]))
                stko = postp.tile([128, NH, CO], F32)
                nc.gpsimd.ap_gather(stko[:], stkrep[:], stkidx[:], channels=128,
                                    num_elems=NSTK, d=CO, num_idxs=NH)
                ka = stko[:, :, :]
                nc.sync.dma_start(
                    bass.AP(stk_o, 0, [[NH * CO, B], [1, NH * CO]]),
                    _ap(ka, 0, [[16 * ka.ap[0][0], B], [1, NH * CO]]))

                spall = postp.tile([128, 2, B * NSTK], F32)
                for h in range(2):
                    for b in range(B):
                        nc.sync.dma_start(
                            spall[:, h, b * NSTK:(b + 1) * NSTK],
                            sparse_d[b, h * 128:(h + 1) * 128, :])
                for h in range(2):
                    spo = wp.tile([128, B * NH, 1], F32, tag="spo")
                    nc.gpsimd.ap_gather(
                        spo[:], spall[:, h, :].rearrange("p (bs u) -> p bs u", u=1),
                        spidx[:], channels=128, num_elems=B * NSTK, d=1, num_idxs=B * NH)
                    nc.sync.dma_start(
                        bass.AP(sparse_o, h * 128 * NH,
                                [[NH, 128], [SP * NH, B], [1, NH]]),
                        spo[:, :, 0])

            # ---- readback prefetch + BN scale/shift + fused GELU + output ----
            with tc.tile_pool(name="yfin", bufs=1) as yfp, \
                 tc.tile_pool(name="ych", bufs=16) as ycp:
                yfin = yfp.tile([128, 2, B * NH * T], F32)
                ychs = []
                for h in range(2):
                    for g in range(16):
                        ych = ycp.tile([128, 512], BF16, tag="ych")
                        nc.sync.dma_start(
                            ych[:], y_hbm[h * 128:(h + 1) * 128, g * 512:(g + 1) * 512])
                        ychs.append(ych)

                scale = pp.tile([128, 2], F32)
                shift = pp.tile([128, 2], F32)
                mwork = pp.tile([128, 8], F32)
                for h in range(2):
                    mean = mwork[:, 4 * h:4 * h + 1]
                    var = mwork[:, 4 * h + 1:4 * h + 2]
                    std = mwork[:, 4 * h + 2:4 * h + 3]
                    msq = mwork[:, 4 * h + 3:4 * h + 4]
                    nc.vector.tensor_scalar_mul(mean, gstats[:, h:h + 1], 1.0 / M_GLOBAL)
                    nc.vector.tensor_scalar_mul(var, gstats[:, 2 + h:3 + h], 1.0 / M_GLOBAL)
                    nc.vector.tensor_tensor(msq, mean, mean, ALU.mult)
                    nc.vector.tensor_tensor(var, var, msq, ALU.subtract)
                    nc.vector.tensor_scalar_add(var, var, BN_EPS)
                    nc.scalar.sqrt(std, var)
                    nc.vector.reciprocal(std, std)
                    nc.vector.tensor_tensor(scale[:, h:h + 1], std, gam_sb[:, h:h + 1], ALU.mult)
                    nc.vector.scalar_tensor_tensor(
                        shift[:, h:h + 1], mean, -1.0, scale[:, h:h + 1], ALU.mult, ALU.mult)
                    nc.vector.tensor_tensor(
                        shift[:, h:h + 1], shift[:, h:h + 1], bet_sb[:, h:h + 1], ALU.add)

                for h in range(2):
                    for g in range(16):
                        ych = ychs[h * 16 + g]
                        yh = yfin[:, h, :]
                        dst = _ap(yh, g * 2 * T,
                                  [list(yh.ap[0]), [NH * T, B], [T, 2], [1, T]])
                        nc.scalar.activation(
                            dst, ych[:], ACTF.Gelu_apprx_tanh,
                            bias=shift[:, h:h + 1], scale=scale[:, h:h + 1])
                for h in range(2):
                    for b in range(B):
                        hw = NH * T // 2
                        nc.sync.dma_start(
                            dense_o[b, h * 128:(h + 1) * 128, :hw],
                            yfin[:, h, b * NH * T:b * NH * T + hw])
                        nc.sync.dma_start(
                            dense_o[b, h * 128:(h + 1) * 128, hw:],
                            yfin[:, h, b * NH * T + hw:(b + 1) * NH * T])

    nc.finalize()
    return nc


def kernel(sparse_fea, dense_fea, stk_coor, conv_w, conv_b, bn_gamma, bn_beta):
    if "nc" not in _CACHE:
        _CACHE["nc"] = _build()
    nc = _CACHE["nc"]

    sparse_fea = np.asarray(sparse_fea, np.float32)
    dense_fea = np.asarray(dense_fea, np.float32)
    stk_coor = np.asarray(stk_coor, np.float32)
    conv_w = np.asarray(conv_w, np.float32)
    conv_b = np.asarray(conv_b, np.float32)
    bn_gamma = np.asarray(bn_gamma, np.float32)
    bn_beta = np.asarray(bn_beta, np.float32)

    in_maps = []
    for c in range(8):
        s = slice(c * B, (c + 1) * B)
        in_maps.append({
            "sparse_fea": sparse_fea[s], "dense_fea": dense_fea[s],
            "stk_coor": stk_coor[s], "conv_w": conv_w, "conv_b": conv_b,
            "bn_gamma": bn_gamma, "bn_beta": bn_beta,
        })
    res = run_bass_kernel_spmd(nc, in_maps, core_ids=list(range(8)),
                               **_CACHE.get("runkw", {}))
    _CACHE["last_result"] = res
    sp = np.concatenate([r["sparse_out"] for r in res.results], 0)
    dn = np.concatenate([r["dense_out"] for r in res.results], 0)
    st = np.concatenate([r["stk_out"] for r in res.results], 0)
    return sp, dn, st
